# revision 44
# baseline (speedup 1.0000x reference)
"""NRI-style GNN encoder (gnn_message_passing) on 8 Trainium2 NeuronCores.

Data-parallel over batch: core b computes batch element b end-to-end.

Structure (v2 — square-grid pass 1):
  - Pass 1 runs on the full N x N "square" edge grid (16384 cols incl.
    diagonal, vs E=16256 compact).  The edge-MLP1 pre-activation is then an
    outer sum u1T[:,r] + v1T[:,s] — one broadcast-AP DVE op per pair, no
    gather matmuls.
  - Compact edge chunk m (128 edges) == square columns 129m+1 .. 129m+129
    (contiguous window), so pass 2 reads the SBUF-resident square ze1 with a
    plain strided AP: no DRAM spill, no transposes.
  - Aggregation: DMA-xbar transposes of ze1 slices + free-size-1 matmuls
    against (1 - I) columns accumulate agg[feat, node] in PSUM.
  - ELU(+1): z = min(exp(y), max(y+1, 1)) with one ACT Exp per layer
    (biases folded into u-columns or applied via ACT bias / TSPtr scalar),
    the linear branch as tensor_scalar, min as tensor_tensor; a column
    split moves part of the psum-sourced branches to the GpSimd engine.
"""

import os
import sys

for _p in ("/opt/trn_rl_repo",):
    if _p not in sys.path:
        sys.path.insert(0, _p)

import numpy as np
import ml_dtypes

import concourse.bass as bass
import concourse.tile as tile
from concourse import bacc, mybir
from concourse.bass_utils import run_bass_kernel_spmd

DT = mybir.dt
AF = mybir.ActivationFunctionType
ALU = mybir.AluOpType

B, N, T, D, H, NE = 8, 128, 49, 4, 256, 2
E = N * (N - 1)          # 16256
ESQ = N * N              # 16384
F = T * D                # 196
MACRO = 512              # compact edges per pass-2 macro
PAIR = 1024              # square cols per pass-1 unit

# Pool can only run tensor_scalar on SBUF operands (TT/STT/psum fail
# neuronxcc codegen).  psum ELU tails use z = max(min(t,1), y+b+1):
#   c = min(t,1): TS on sbuf fp16 -> Pool (or DVE at 4x)
#   z = max(c, y+b+1): STT from psum -> DVE, or ACT-Identity + TS/TT form
POOL_C_OF_16 = 8   # c-ops on Pool, of 16
ACTC_OF_16 = 5      # ACT-copy-form ops (w = Identity(ps+b)), of 16
ACTC_P1B = 8
S3_POOL8 = 8


def _mk_layout(entries):
    out, c = {}, 0
    for name, w in entries:
        out[name] = (c, w)
        c += w
    return out, c

PK32, C32 = _mk_layout([
    ("ey32", 128), ("wn1a", 256), ("wn1b", 256), ("wn1l2", 512),
    ("nbs", 1024),
    ("wn2l1", 512), ("wn2l2", 512), ("a2s", 512), ("b2s", 512),
    ("bos", 16), ("b2e", 2), ("b2p1", 2), ("b4e", 2), ("b4p1", 2),
    ("ones1", 128), ("be3r", 256),
])
PK32_SPLIT = 128 + 256 + 256 + 512 + 1024
PK16, C16 = _mk_layout([
    ("we1l2", 512), ("c2s", 512), ("we2l2", 512),
    ("a1s", 512), ("b1s", 512), ("onemi", 128),
    ("ones16", 512), ("be1r", 256), ("be4r", 256), ("ows", 4),
])

_PROG_CACHE = {}
LAST_EXEC_NS = None


def _build_program():
    nc = bacc.Bacc(
        "TRN2",
        target_bir_lowering=False,
        debug=False,
        enable_asserts=True,
        num_devices=8,
    )

    f32, f16 = DT.float32, DT.float16
    f8 = DT.float8e4

    def din(name, shape, dt=f32):
        return nc.dram_tensor(name, list(shape), dt, kind="ExternalInput").ap()

    x_in = din("x_nm", [N, F])
    recT = din("recT", [N, E], f8)
    sendT = din("sendT", [N, E], f8)
    pk32 = din("pk32", [128, C32], f32)
    pk16 = din("pk16", [128, C16], f16)

    out_d = nc.dram_tensor("out", [E, NE], f32, kind="ExternalOutput").ap()

    rctr = [0]

    with tile.TileContext(nc) as tc:
        with (
            tc.tile_pool(name="const", bufs=1) as cpool,
            tc.tile_pool(name="rel", bufs=1) as relpool,
            tc.tile_pool(name="zres", bufs=1) as zres,
            tc.tile_pool(name="work", bufs=2) as wk,
        ):
            # ---------- load constants ----------
            def ctile(ap_dram, shape, dt=f32, name="c"):
                t = cpool.tile(shape, dt, name=name)
                nc.sync.dma_start(t[:], ap_dram)
                return t

            x_sb = ctile(x_in, [N, F], name="x_sb")
            p32 = cpool.tile([128, C32], f32, name="p32")
            nc.sync.dma_start(p32[:, :PK32_SPLIT], pk32[:, :PK32_SPLIT])
            p16 = ctile(pk16, [128, C16], f16, name="p16")
            nc.sync.dma_start(p32[:, PK32_SPLIT:], pk32[:, PK32_SPLIT:])

            def c32(name, hview=False):
                c0, w = PK32[name]
                ap = p32[:, c0:c0 + w]
                if hview:
                    ap = ap.rearrange("p (h o) -> p h o", h=2)
                return ap

            def c16(name, hview=False):
                c0, w = PK16[name]
                ap = p16[:, c0:c0 + w]
                if hview:
                    ap = ap.rearrange("p (h o) -> p h o", h=2)
                return ap

            ey32 = c32("ey32")
            wn1a = c32("wn1a")
            wn1b = c32("wn1b")[0:68, :]
            wn1l2 = c32("wn1l2", hview=True)
            wn2l1 = c32("wn2l1", hview=True)
            wn2l2 = c32("wn2l2", hview=True)
            a2s = c32("a2s", hview=True)
            b2s = c32("b2s", hview=True)
            nbs = c32("nbs").rearrange("p (h o) -> p h o", h=4)
            bos = c32("bos")
            b2e = c32("b2e")
            b2p1 = c32("b2p1")
            b4e = c32("b4e")
            b4p1 = c32("b4p1")
            ones1 = c32("ones1")[0:1, :]
            be3r = c32("be3r")[0:1, :]

            we1l2 = c16("we1l2", hview=True)
            c2s = c16("c2s", hview=True)
            we2l2 = c16("we2l2", hview=True)
            a1s = c16("a1s", hview=True)
            b1s = c16("b1s", hview=True)
            onemi = c16("onemi")
            ones16 = c16("ones16")[0:1, :]
            be1r = c16("be1r")[0:1, :]
            be4r = c16("be4r")[0:1, :]
            ows = c16("ows", hview=True)

            # rel matrices (pass 2 only); loaded in slices to overlap pass 1
            recT_sb = relpool.tile([128, E], f8, name="recT_sb")
            sendT_sb = relpool.tile([128, E], f8, name="sendT_sb")
            bounds = [0, 2048, 4096, 8192, 12288, E]
            for c0, c1 in zip(bounds[:-1], bounds[1:]):
                nc.scalar.dma_start(recT_sb[:, c0:c1], recT[:, c0:c1])
                nc.scalar.dma_start(sendT_sb[:, c0:c1], sendT[:, c0:c1])

            # resident square ze1 [feat-half(p), half, col]
            ze1 = zres.tile([128, 2, ESQ], f16, name="ze1")

            # PSUM pool for node stages + pass 1; closed before pass 2
            l2_ps_cm = tc.tile_pool(name="l2_ps", bufs=2, space="PSUM")
            l2_ps = l2_ps_cm.__enter__()

            def psum_elu_tail(dst_ap, ps_ap, t_ap, r_tile_ap, b_add,
                              site, pool8, actc=ACTC_OF_16):
                """z = max(min(t,1), y+b+1) from psum y (exact ELU+1)."""
                k = rctr[0]
                rctr[0] += 1
                ceng = nc.gpsimd if (k % 16) < pool8 else nc.vector
                ceng.tensor_scalar(r_tile_ap, t_ap, 1.0, 1.0,
                                   ALU.min, ALU.mult)
                if ((k + 7) % 16) < actc:
                    # w = y+b+1 via ACT Identity, then z = max(c, w) on DVE
                    w = wk.tile([128, 2 * MACRO], f16, name="w_ab",
                                tag="w_ab", bufs=2)
                    nfree = ps_ap.free_size()
                    nc.scalar.activation(w[:, :nfree], ps_ap, AF.Identity,
                                         bias=b_add)
                    nc.vector.tensor_tensor(dst_ap, r_tile_ap, w[:, :nfree],
                                            ALU.max)
                else:
                    nc.vector.scalar_tensor_tensor(dst_ap, ps_ap, b_add,
                                                   r_tile_ap, ALU.add,
                                                   ALU.max)

            # ---------- node-stage helpers (fp32, f32r matmuls) ----------
            def node_mm(lhsT_tile, rhs_tile, nh=2, brow=None, rows=()):
                if brow is not None:
                    rows = ((ones1, brow),) + tuple(rows)
                ps = l2_ps.tile([128, PAIR], f32, name="ps_n", tag="l2")
                for fh in range(nh):
                    nc.tensor.matmul(ps[:, :256],
                                     lhsT_tile[:, fh], rhs_tile[:, fh],
                                     start=(fh == 0),
                                     stop=(fh == nh - 1 and not rows))
                for i, (lr, rr) in enumerate(rows):
                    nc.tensor.matmul(ps[:, :256], lr, rr,
                                     start=False, stop=(i == len(rows) - 1))
                return ps

            def add_bias_sbuf(ps, btile, name):
                y = wk.tile([128, 256], f32, name=name, tag="y_n")
                nc.vector.tensor_tensor(y[:], ps[:, :256], btile, ALU.add)
                return y

            def elu_N(y_sb, out_name):
                t = wk.tile([128, 256], f32, name="t_n", tag="t_n")
                nc.scalar.activation(t[:], y_sb, AF.Exp)
                r = wk.tile([128, 256], f32, name="r_n", tag="r_n")
                nc.vector.tensor_scalar(r[:], y_sb, 1.0, 1.0, ALU.add, ALU.max)
                z = cpool.tile([128, 256], f32, name=out_name)
                nc.vector.tensor_tensor(z[:], t[:], r[:], ALU.min)
                return z

            def tpose_nf(src_sb, out_name, dt_out=f32):
                """[128n, 256f] sbuf -> [128f-local, 2(fh), 128n] sbuf."""
                ps = l2_ps.tile([128, PAIR], f32, name="ps_tp", tag="l2")
                for fh in range(2):
                    nc.tensor.transpose(ps[:, fh * 128:(fh + 1) * 128],
                                        src_sb[:, fh * 128:(fh + 1) * 128],
                                        ey32)
                t = cpool.tile([128, 2, 128], dt_out, name=out_name)
                if dt_out == f32:
                    nc.vector.tensor_copy(t[:].rearrange("p a b -> p (a b)"),
                                          ps[:, :256])
                else:
                    nc.scalar.copy(t[:].rearrange("p a b -> p (a b)"),
                                   ps[:, :256])
                return t

            # ---------- node stage 1 ----------
            ps_x = l2_ps.tile([128, PAIR], f32, name="ps_x", tag="l2")
            nc.tensor.transpose(ps_x[:, 0:128], x_sb[:, 0:128], ey32)
            nc.tensor.transpose(ps_x[0:68, 128:256], x_sb[:, 128:196], ey32)
            xt0 = cpool.tile([128, 128], f32, name="xt0")
            nc.vector.tensor_copy(xt0[:], ps_x[:, 0:128])
            xt1 = cpool.tile([68, 128], f32, name="xt1")
            nc.vector.tensor_copy(xt1[:], ps_x[0:68, 128:256])

            ps1 = l2_ps.tile([128, PAIR], f32, name="ps1", tag="l2")
            nc.tensor.matmul(ps1[:, :256], xt0[:], wn1a[:],
                             start=True, stop=False)
            nc.tensor.matmul(ps1[:, :256], xt1[:], wn1b[:],
                             start=False, stop=True)
            y1 = add_bias_sbuf(ps1, nbs[:, 0, :], "y1")
            zh1a = elu_N(y1[:], "zh1a")
            zh1aT = tpose_nf(zh1a, "zh1aT")

            ps2 = node_mm(zh1aT, wn1l2)
            y2 = add_bias_sbuf(ps2, nbs[:, 1, :], "y2")
            zh1 = elu_N(y2[:], "zh1")
            zh1T16 = tpose_nf(zh1, "zh1T16", dt_out=f16)

            # u1T/v1T: [feat-local(p), half, node] fp16, be1 folded into u1T
            def edge_lhs_T(wsq, name, brow=None):
                t = cpool.tile([128, 2, 128], f16, name=name)
                for oh in range(2):
                    ps = l2_ps.tile([128, PAIR], f32, name="ps_uv", tag="l2")
                    for fh in range(2):
                        nc.tensor.matmul(
                            ps[:, :128],
                            wsq[:, fh, oh * 128:(oh + 1) * 128],
                            zh1T16[:, fh],
                            start=(fh == 0),
                            stop=(fh == 1 and brow is None))
                    if brow is not None:
                        nc.tensor.matmul(
                            ps[:, :128], brow[:, oh * 128:(oh + 1) * 128],
                            ones16[:, 0:128], start=False, stop=True)
                    nc.scalar.copy(t[:, oh, :], ps[:, :128])
                return t

            u1t = edge_lhs_T(a1s, "u1t", brow=be1r)
            v1t = edge_lhs_T(b1s, "v1t")

            # ---------- pass 1 over the square grid ----------
            npairs = ESQ // PAIR  # 16

            def p1_stageA(i):
                """outer-sum + ELU -> z1a [128, 2, PAIR] fp16."""
                c0 = i * PAIR
                b0 = c0 // 128  # first block index (8 per pair)
                y = wk.tile([128, 2, PAIR], f16, name="y1a", tag="y1a",
                            bufs=2)
                yv = y[:].rearrange("p h (b n) -> p h b n", n=128)
                vv = v1t[:].rearrange("p h (o n) -> p h o n", o=1) \
                    .broadcast_to((128, 2, 8, 128))
                uv = u1t[:, :, b0:b0 + 8].rearrange(
                    "p h (b o) -> p h b o", o=1).broadcast_to((128, 2, 8, 128))
                nc.vector.tensor_tensor(yv, vv, uv, ALU.add)
                yf = y[:].rearrange("p h c -> p (h c)")
                t = wk.tile([128, 2 * PAIR], f16, name="t1a", tag="t1a",
                            bufs=2)
                nc.scalar.activation(t[:], yf, AF.Exp)
                r = wk.tile([128, 2 * PAIR], f16, name="r1a", tag="r1a",
                            bufs=2)
                s3eng = nc.gpsimd if (i % 16) < S3_POOL8 else nc.vector
                s3eng.tensor_scalar(r[:], yf, 1.0, 1.0, ALU.add, ALU.max)
                z = wk.tile([128, 2, PAIR], f16, name="z1a", tag="z1a",
                            bufs=2)
                nc.vector.tensor_tensor(z[:].rearrange("p h c -> p (h c)"),
                                        t[:], r[:], ALU.min)
                return z

            def p1_stageB(i, z1a):
                """edge-MLP1 layer 2 + ELU -> resident ze1; DMA transposes."""
                c0 = i * PAIR
                for oh in range(2):
                    ps = l2_ps.tile([128, PAIR], f32, name="ps_l2",
                                    tag="l2")
                    for hv in range(2):
                        for fh in range(2):
                            nc.tensor.matmul(
                                ps[:, hv * 512:(hv + 1) * 512],
                                we1l2[:, fh, oh * 128:(oh + 1) * 128],
                                z1a[:, fh, hv * 512:(hv + 1) * 512],
                                start=(fh == 0), stop=(fh == 1))
                    t = wk.tile([128, PAIR], f16, name="t1b", tag="t1b",
                                bufs=4)
                    nc.scalar.activation(t[:], ps[:], AF.Exp,
                                         bias=b2e[:, oh:oh + 1])
                    r = wk.tile([128, PAIR], f16, name="r1b", tag="r1b",
                                bufs=4)
                    psum_elu_tail(ze1[:, oh, c0:c0 + PAIR], ps[:], t[:],
                                  r[:], b2p1[:, oh:oh + 1], "p1b", 16)
                zt = wk.tile([128, 2, 8, 128], f16, name="zt", tag="zt",
                             bufs=2)
                for fh in range(2):
                    nc.sync.dma_start_transpose(
                        zt[:, fh], ze1[:, fh, c0:c0 + PAIR])
                return zt

            def p1_stageC(i, zt, aggp):
                b0 = i * 8
                for j in range(8):
                    for fh in range(2):
                        nc.tensor.matmul(
                            aggp[:, fh, b0 + j:b0 + j + 1],
                            zt[:, fh, j, :],
                            onemi[:, b0 + j:b0 + j + 1],
                            start=True, stop=True,
                            skip_group_check=True)

            with tc.tile_pool(name="agg_ps", bufs=1, space="PSUM") as agg_ps:
                aggp = agg_ps.tile([128, 2, 128], f32, name="aggp")
                recs = []
                for i in range(npairs):
                    z1a = p1_stageA(i)
                    recs.append([i, z1a, None])
                    if i >= 1:
                        r = recs[i - 1]
                        r[2] = p1_stageB(r[0], r[1])
                        p1_stageC(r[0], r[2], aggp)
                r = recs[-1]
                r[2] = p1_stageB(r[0], r[1])
                p1_stageC(r[0], r[2], aggp)

                # ---------- node stage 2 ----------
                aggT = cpool.tile([128, 2, 128], f32, name="aggT")
                nc.scalar.copy(aggT[:].rearrange("p a b -> p (a b)"),
                               aggp[:].rearrange("p a b -> p (a b)"))

            ps3 = node_mm(aggT, wn2l1)
            y3 = add_bias_sbuf(ps3, nbs[:, 2, :], "y3")
            zh2a = elu_N(y3[:], "zh2a")
            zh2aT = tpose_nf(zh2a, "zh2aT")

            ps4 = node_mm(zh2aT, wn2l2)
            y4 = add_bias_sbuf(ps4, nbs[:, 3, :], "y4")
            zh2 = elu_N(y4[:], "zh2")
            zh2T = tpose_nf(zh2, "zh2T")

            def copy16(ps, name):
                u = cpool.tile([128, 256], f16, name=name)
                nc.scalar.copy(u[:], ps[:, :256])
                return u

            u2 = copy16(node_mm(zh2T, a2s, brow=be3r), "u2")
            v2 = copy16(node_mm(zh2T, b2s), "v2")

            l2_ps_cm.__exit__(None, None, None)

            # ---------- pass 2 over compact edges ----------
            offs = list(range(0, E, MACRO))

            def sq_window(hh, off, L):
                """ze1 cols for compact edges [off, off+L) as strided 3D AP."""
                nch = L // 128
                c0 = off // 128
                base = 129 * c0 + 1
                return ze1[:, hh, base:base + 129 * nch].rearrange(
                    "p (c w) -> p c w", w=129)[:, :, 0:128]

            def elu_edge(ps, L, out_ap2, tname, rname, site, pool8):
                """ELU(+1) on psum [128, 2, L-of-MACRO] -> out [128,2,L]."""
                t = wk.tile([128, 2 * MACRO], f16, name=tname, tag="t1b",
                            bufs=4)
                r = wk.tile([128, 2 * MACRO], f16, name=rname, tag="r1b",
                            bufs=4)
                if L == MACRO:
                    psf = ps[:].rearrange("p h c -> p (h c)")
                    nc.scalar.activation(t[:], psf, AF.Exp)
                    psum_elu_tail(out_ap2.rearrange("p h c -> p (h c)"),
                                  psf, t[:], r[:], 1.0, site, pool8)
                else:
                    for fh in range(2):
                        nc.scalar.activation(t[:, fh * MACRO:fh * MACRO + L],
                                             ps[:, fh, :L], AF.Exp)
                        psum_elu_tail(out_ap2[:, fh, :L], ps[:, fh, :L],
                                      t[:, fh * MACRO:fh * MACRO + L],
                                      r[:, fh * MACRO:fh * MACRO + L],
                                      1.0, site, pool8)

            def p2_stageA(off, L, pps):
                ze2a = wk.tile([128, 2, MACRO], f16, name="ze2a", tag="ze2a",
                               bufs=3)
                ps = pps.tile([128, 2, MACRO], f32, name="ps_p2", tag="p2ps",
                              bufs=2)
                for fh in range(2):
                    for hh in range(2):
                        nc.tensor.matmul(
                            ps[:, fh, :L],
                            c2s[:, hh, fh * 128:(fh + 1) * 128],
                            sq_window(hh, off, L),
                            start=(hh == 0), stop=False)
                    nc.tensor.matmul(
                        ps[:, fh, :L], u2[:, fh * 128:(fh + 1) * 128],
                        recT_sb[:, off:off + L], start=False, stop=False)
                    nc.tensor.matmul(
                        ps[:, fh, :L], v2[:, fh * 128:(fh + 1) * 128],
                        sendT_sb[:, off:off + L], start=False, stop=True)
                elu_edge(ps, L, ze2a[:], "t2a", "r2a", "p2a", 0)
                return ze2a

            def p2_stageB(off, L, ze2a, bps, out_ps):
                ze2 = wk.tile([128, 2, MACRO], f16, name="ze2", tag="ze2a",
                              bufs=3)
                for oh in range(2):
                    ps = bps.tile([128, MACRO], f32, name="ps_l2b",
                                  tag="p2psB")
                    for fh in range(2):
                        nc.tensor.matmul(
                            ps[:, :L],
                            we2l2[:, fh, oh * 128:(oh + 1) * 128],
                            ze2a[:, fh, :L],
                            start=(fh == 0), stop=(fh == 1))
                    t = wk.tile([128, MACRO], f16, name="t2b", tag="t1b",
                                bufs=4)
                    nc.scalar.activation(t[:, :L], ps[:, :L], AF.Exp,
                                         bias=b4e[:, oh:oh + 1])
                    r = wk.tile([128, MACRO], f16, name="r2b", tag="r1b",
                                bufs=4)
                    psum_elu_tail(ze2[:, oh, :L], ps[:, :L], t[:, :L],
                                  r[:, :L], b4p1[:, oh:oh + 1], "p2b", 8)

                nsub = L // 128
                op = out_ps.tile([128, 4, NE], f32, name="op", tag="op")
                for j in range(nsub):
                    for oh in range(2):
                        nc.tensor.matmul(
                            op[:, j, :],
                            ze2[:, oh, j * 128:(j + 1) * 128],
                            ows[:, oh, :],
                            start=(oh == 0), stop=(oh == 1))
                osb = wk.tile([128, 4 * NE], f32, name="osb", tag="osb")
                nc.vector.tensor_tensor(
                    osb[:, :nsub * NE],
                    op[:].rearrange("p a b -> p (a b)")[:, :nsub * NE],
                    bos[:, :nsub * NE], ALU.add)
                nc.sync.dma_start(
                    out_d[off:off + L, :].rearrange("(j p) c -> p j c", p=128),
                    osb[:, :nsub * NE].rearrange("p (j c) -> p j c", c=NE))

            with (
                tc.tile_pool(name="p2_ps", bufs=2, space="PSUM") as p2_ps,
                tc.tile_pool(name="p2b_ps", bufs=3, space="PSUM") as p2b_ps,
                tc.tile_pool(name="out_ps", bufs=1, space="PSUM") as out_ps,
            ):
                recs2 = []
                for off in offs:
                    L = min(MACRO, E - off)
                    ze2a = p2_stageA(off, L, p2_ps)
                    recs2.append((off, L, ze2a))
                    if len(recs2) >= 3:
                        o2, L2, z2 = recs2[-3]
                        p2_stageB(o2, L2, z2, p2b_ps, out_ps)
                for o2, L2, z2 in recs2[-2:]:
                    p2_stageB(o2, L2, z2, p2b_ps, out_ps)

    nc.compile()
    return nc


def _prep_inputs(inputs):
    """Host-side constant preprocessing -> shared in_map (all cores)."""
    f = lambda a: np.ascontiguousarray(np.asarray(a, dtype=np.float32))
    rec, send = f(inputs["rec_rel"]), f(inputs["send_rel"])
    cs = lambda w: w.sum(axis=0)

    n1w1, n1b1 = f(inputs["n1w1"]), f(inputs["n1b1"])
    n1w2, n1b2 = f(inputs["n1w2"]), f(inputs["n1b2"])
    e1w1, e1b1 = f(inputs["e1w1"]), f(inputs["e1b1"])
    e1w2, e1b2 = f(inputs["e1w2"]), f(inputs["e1b2"])
    n2w1, n2b1 = f(inputs["n2w1"]), f(inputs["n2b1"])
    n2w2, n2b2 = f(inputs["n2w2"]), f(inputs["n2b2"])
    e2w1, e2b1 = f(inputs["e2w1"]), f(inputs["e2b1"])
    e2w2, e2b2 = f(inputs["e2w2"]), f(inputs["e2b2"])
    ow, ob = f(inputs["ow"]), f(inputs["ob"])

    A1, B1 = e1w1[:256], e1w1[256:]
    A2, B2, C2 = e2w1[:256], e2w1[256:512], e2w1[512:]

    A1h = A1.astype(np.float16)
    B1h = B1.astype(np.float16)
    e1w2_h = e1w2.astype(np.float16)
    C2_h = C2.astype(np.float16)
    e2w2_h = e2w2.astype(np.float16)
    ow_h = ow.astype(np.float16)

    # z+1 storage: "-1" of each consumed activation folds into biases
    be1 = e1b1 - cs(A1h.astype(np.float32)) - cs(B1h.astype(np.float32))
    be2 = e1b2 - cs(e1w2_h.astype(np.float32))
    be3 = e2b1 - cs(A2) - cs(B2) - cs(C2_h.astype(np.float32))
    be4 = e2b2 - cs(e2w2_h.astype(np.float32))
    ob_adj = ob - cs(ow_h.astype(np.float32))

    indeg = rec.sum(axis=0)  # [N] (=127)
    nbias = np.zeros((128, 4, 256), np.float32)
    nbias[:, 0, :] = n1b1[None, :]
    nbias[:, 1, :] = (n1b2 - cs(n1w2))[None, :]
    nbias[:, 2, :] = n2b1[None, :] - indeg[:, None] * cs(n2w1)[None, :]
    nbias[:, 3, :] = (n2b2 - cs(n2w2))[None, :]

    bout = np.tile(ob_adj[None, :], (128, 8)).astype(np.float32)

    def sqh(w):  # [256, x] -> [128, 2*x] partition-major halves
        return np.ascontiguousarray(
            w.reshape(2, 128, -1).transpose(1, 0, 2).reshape(128, -1))

    pk32 = np.zeros((128, C32), np.float32)
    def put32(name, arr):
        c0, w = PK32[name]
        pk32[:arr.shape[0], c0:c0 + w] = arr
    put32("ey32", np.eye(128, dtype=np.float32))
    put32("wn1a", n1w1[:128])
    put32("wn1b", n1w1[128:])
    put32("wn1l2", sqh(n1w2))
    put32("wn2l1", sqh(n2w1))
    put32("wn2l2", sqh(n2w2))
    put32("a2s", sqh(A2))
    put32("b2s", sqh(B2))
    put32("nbs", nbias.reshape(128, -1))
    put32("bos", bout)
    put32("b2e", np.ascontiguousarray(be2.reshape(2, 128).T))
    put32("b2p1", np.ascontiguousarray((be2 + 1.0).reshape(2, 128).T))
    put32("b4e", np.ascontiguousarray(be4.reshape(2, 128).T))
    put32("b4p1", np.ascontiguousarray((be4 + 1.0).reshape(2, 128).T))
    c0, w = PK32["ones1"]; pk32[0, c0:c0 + w] = 1.0
    c0, w = PK32["be3r"]; pk32[0, c0:c0 + w] = be3

    pk16 = np.zeros((128, C16), np.float16)
    def put16(name, arr):
        c0, w = PK16[name]
        pk16[:arr.shape[0], c0:c0 + w] = arr
    put16("we1l2", sqh(e1w2_h.astype(np.float32)).astype(np.float16))
    put16("c2s", sqh(C2_h.astype(np.float32)).astype(np.float16))
    put16("we2l2", sqh(e2w2_h.astype(np.float32)).astype(np.float16))
    put16("a1s", sqh(A1h.astype(np.float32)).astype(np.float16))
    put16("b1s", sqh(B1h.astype(np.float32)).astype(np.float16))
    put16("onemi", (1.0 - np.eye(128)).astype(np.float16))
    c0, w = PK16["ones16"]; pk16[0, c0:c0 + w] = 1.0
    c0, w = PK16["be1r"]; pk16[0, c0:c0 + w] = be1.astype(np.float16)
    c0, w = PK16["be4r"]; pk16[0, c0:c0 + w] = be4.astype(np.float16)
    put16("ows", sqh(ow_h.astype(np.float32)).astype(np.float16))

    shared = dict(
        recT=np.ascontiguousarray(rec.T.astype(ml_dtypes.float8_e4m3)),
        sendT=np.ascontiguousarray(send.T.astype(ml_dtypes.float8_e4m3)),
        pk32=pk32, pk16=pk16,
    )
    return shared


def kernel(**inputs):
    global LAST_EXEC_NS
    if "prog" not in _PROG_CACHE:
        _PROG_CACHE["prog"] = _build_program()
    nc = _PROG_CACHE["prog"]

    shared = _prep_inputs(inputs)
    x = np.asarray(inputs["x"], dtype=np.float32)
    in_maps = []
    for b in range(B):
        m = dict(shared)
        m["x_nm"] = np.ascontiguousarray(x[b].reshape(N, F))
        in_maps.append(m)

    trace = os.environ.get("KERNEL_TRACE", "0") == "1"
    try:
        res = run_bass_kernel_spmd(nc, in_maps, core_ids=list(range(8)),
                                   trace=trace)
    except ModuleNotFoundError:
        res = run_bass_kernel_spmd(nc, in_maps, core_ids=list(range(8)),
                                   trace=False)
    if trace and res.exec_time_ns is not None:
        LAST_EXEC_NS = res.exec_time_ns
        print(f"HW exec time: {res.exec_time_ns} ns "
              f"(mean {res.mean_exec_time_ns} ns, "
              f"slowest core {res.max_exec_time_core_id})")

    out = np.stack([res.results[b]["out"] for b in range(B)], axis=0)
    return out.astype(np.float32)
